# revision 3
# baseline (speedup 1.0000x reference)
"""Trainium2 Bass kernel for nn_AttentionModel_23304492548756.

Sharding: data-parallel over batch for the 6-layer transformer (16 samples ->
2 per core x 8 cores, weights replicated, bf16 matmuls with fp32 PSUM
accumulation), then the large end-layer matmul [16, C*S] @ We is sharded over
its contraction dim (each core takes 1/8 of the C*S rows of We); the partial
[16, O] outputs are summed on the host (cheaper than an on-device collective).
"""
import math
from contextlib import ExitStack

import numpy as np
import ml_dtypes

import concourse.bass as bass
import concourse.tile as tile
from concourse import bacc, mybir
from concourse.bass import ts
from concourse.masks import make_identity
from concourse.bass_utils import run_bass_kernel_spmd

FP32 = mybir.dt.float32
BF16 = mybir.dt.bfloat16
AF = mybir.ActivationFunctionType
ALU = mybir.AluOpType

B = 16
C = 512
S = 512
H = 8
L = 6
FF = 2048
O = 512
DH = C // H
EPS = 1e-5
NT = 4           # C/128 = S/128 tiles
NFT = FF // 128  # 16
N_CORES = 8
SPC = B // N_CORES          # samples per core
KSH = C * S // N_CORES      # end-layer contraction shard


def build_transformer(n_layers=L, n_samples=SPC, n_cores=N_CORES,
                      use_g1=False, use_beta1=False, use_g2=False, use_beta2=False,
                      use_bo=False, use_b1=False, use_b2=False):
    NL, NS = n_layers, n_samples
    nc = bacc.Bacc("TRN2", target_bir_lowering=False, debug=False,
                   num_devices=n_cores)

    h0_d = nc.dram_tensor("h0", [NS, S, C], FP32, kind="ExternalInput").ap()
    wq_d = nc.dram_tensor("wq", [NL, C, C], BF16, kind="ExternalInput").ap()
    wk_d = nc.dram_tensor("wk", [NL, C, C], BF16, kind="ExternalInput").ap()
    wv_d = nc.dram_tensor("wv", [NL, C, C], BF16, kind="ExternalInput").ap()
    wo_d = nc.dram_tensor("wo", [NL, C, C], BF16, kind="ExternalInput").ap()
    w1_d = nc.dram_tensor("w1", [NL, C, FF], BF16, kind="ExternalInput").ap()
    w2_d = nc.dram_tensor("w2", [NL, FF, C], BF16, kind="ExternalInput").ap()
    b1_d = nc.dram_tensor("b1", [NL, FF], FP32, kind="ExternalInput").ap() if use_b1 else None
    vec_d = {}
    for name, used in (("bo", use_bo), ("b2", use_b2), ("g1", use_g1),
                       ("beta1", use_beta1), ("g2", use_g2), ("beta2", use_beta2)):
        if used:
            vec_d[name] = nc.dram_tensor(name, [NL, C], FP32, kind="ExternalInput").ap()
    hout_d = nc.dram_tensor("hout", [NS, S, C], BF16, kind="ExternalOutput").ap()

    with tile.TileContext(nc) as tc, ExitStack() as ctx:
        const_p = ctx.enter_context(tc.tile_pool(name="const", bufs=1))
        wpool = ctx.enter_context(tc.tile_pool(name="w", bufs=2))
        bias_p = ctx.enter_context(tc.tile_pool(name="biasv", bufs=2))
        hsc_p = ctx.enter_context(tc.tile_pool(name="hsc", bufs=10))
        hb_p = ctx.enter_context(tc.tile_pool(name="hb", bufs=6))
        hcs_p = ctx.enter_context(tc.tile_pool(name="hcs", bufs=8))
        qkv_p = ctx.enter_context(tc.tile_pool(name="qkv", bufs=4))
        e_p = ctx.enter_context(tc.tile_pool(name="e", bufs=9))
        at_p = ctx.enter_context(tc.tile_pool(name="at", bufs=9))
        ot_p = ctx.enter_context(tc.tile_pool(name="ot", bufs=5))
        f1_p = ctx.enter_context(tc.tile_pool(name="f1", bufs=18))
        z_p = ctx.enter_context(tc.tile_pool(name="z", bufs=4))
        rzb_p = ctx.enter_context(tc.tile_pool(name="rzb", bufs=3))
        st_p = ctx.enter_context(tc.tile_pool(name="st", bufs=16))
        out_p = ctx.enter_context(tc.tile_pool(name="out", bufs=4))

        ps_mm = ctx.enter_context(tc.tile_pool(name="ps_mm", bufs=2, space="PSUM"))
        ps_sc = ctx.enter_context(tc.tile_pool(name="ps_sc", bufs=4, space="PSUM"))
        ps_tr = ctx.enter_context(tc.tile_pool(name="ps_tr", bufs=2, space="PSUM"))

        ident = const_p.tile([128, 128], BF16)
        make_identity(nc, ident[:])
        eps_t = const_p.tile([128, 1], FP32)
        nc.vector.memset(eps_t[:], EPS)
        ones_b = const_p.tile([128, 1], BF16)
        nc.vector.memset(ones_b[:], 1.0)

        hsc = [[None] * NT for _ in range(NS)]
        hcs = [[None] * NT for _ in range(NS)]

        def transpose_to_cs(hb_tiles, tag):
            res = []
            for t in range(NT):
                pst = ps_tr.tile([128, S], BF16, tag="tr", name="tr")
                for u in range(NT):
                    nc.tensor.transpose(pst[:, ts(u, 128)],
                                        hb_tiles[u][:, ts(t, 128)], ident[:])
                dst = hcs_p.tile([128, S], BF16, tag=tag, name=tag)
                nc.scalar.copy(dst[:], pst[:])
                res.append(dst)
            return res

        for s in range(NS):
            hbt = []
            for t in range(NT):
                hsc[s][t] = hsc_p.tile([128, C], FP32, tag="hsc", name="hsc")
                nc.sync.dma_start(hsc[s][t][:], h0_d[s, ts(t, 128), :])
                hb = hb_p.tile([128, C], BF16, tag="hb", name="hb")
                nc.gpsimd.tensor_copy(hb[:], hsc[s][t][:])
                hbt.append(hb)
            hcs[s] = transpose_to_cs(hbt, "hcs")

        for l in range(NL):
            wq_sb = wpool.tile([128, NT, C], BF16, tag="wq", name="wq")
            wk_sb = wpool.tile([128, NT, C], BF16, tag="wk", name="wk")
            wv_sb = wpool.tile([128, NT, C], BF16, tag="wv", name="wv")
            wo_sb = wpool.tile([128, NT, C], BF16, tag="wo", name="wo")
            w1_sb = wpool.tile([128, NT, FF], BF16, tag="w1", name="w1", bufs=1)
            w2_sb = wpool.tile([128, NFT, C], BF16, tag="w2", name="w2", bufs=1)
            nc.sync.dma_start(wq_sb[:], wq_d[l].rearrange("(ci p) c -> p ci c", p=128))
            nc.sync.dma_start(wk_sb[:], wk_d[l].rearrange("(ci p) c -> p ci c", p=128))
            nc.sync.dma_start(wv_sb[:], wv_d[l].rearrange("(ci p) c -> p ci c", p=128))
            nc.sync.dma_start(wo_sb[:], wo_d[l].rearrange("(ci p) c -> p ci c", p=128))
            nc.sync.dma_start(w1_sb[:], w1_d[l].rearrange("(ci p) f -> p ci f", p=128))
            nc.sync.dma_start(w2_sb[:], w2_d[l].rearrange("(ft p) c -> p ft c", p=128))
            if use_b1:
                b1_sb = bias_p.tile([128, NFT], FP32, tag="b1", name="b1")
                nc.sync.dma_start(b1_sb[:], b1_d[l].rearrange("(ft p) -> p ft", p=128))
            vec_sb = {}
            for name in vec_d:
                vb = bias_p.tile([128, C], FP32, tag=name, name=name)
                src = bass.AP(tensor=vec_d[name].tensor, offset=l * C,
                              ap=[[0, 128], [1, C]])
                nc.gpsimd.dma_start(vb[:], src)
                vec_sb[name] = vb

            for s in range(NS):
                # ---- QKV ----
                qT, kT, vN = [], [], []
                for t in range(NT):
                    psq = ps_mm.tile([128, C], FP32, tag="mm", name="mm")
                    for ci in range(NT):
                        nc.tensor.matmul(psq[:], wq_sb[:, ci, ts(t, 128)],
                                         hcs[s][ci][:], start=(ci == 0), stop=(ci == NT - 1))
                    qt = qkv_p.tile([128, S], BF16, tag="qT", name="qT")
                    nc.scalar.copy(qt[:], psq[:])
                    qT.append(qt)

                    psk = ps_mm.tile([128, C], FP32, tag="mm", name="mm")
                    for ci in range(NT):
                        nc.tensor.matmul(psk[:], wk_sb[:, ci, ts(t, 128)],
                                         hcs[s][ci][:], start=(ci == 0), stop=(ci == NT - 1))
                    kt = qkv_p.tile([128, S], BF16, tag="kT", name="kT")
                    nc.scalar.copy(kt[:], psk[:])
                    kT.append(kt)

                    psv = ps_mm.tile([128, C], FP32, tag="mm", name="mm")
                    for ci in range(NT):
                        nc.tensor.matmul(psv[:], hcs[s][ci][:, ts(t, 128)],
                                         wv_sb[:, ci, :], start=(ci == 0), stop=(ci == NT - 1))
                    vt = qkv_p.tile([128, C], BF16, tag="v", name="v")
                    nc.vector.tensor_copy(vt[:], psv[:])
                    vN.append(vt)

                # ---- attention (head pairs; rows 0-63 / 64-127 of qT/kT c-tiles) ----
                oT = []
                for j in range(NT):
                    if l == 0:
                        # numerically-safe path: scores in [q, k] layout with
                        # max-subtracted softmax, PE-transpose of attn
                        E = [[None] * NT for _ in range(2)]
                        for qt in range(NT):
                            for sub in range(2):
                                pss = ps_sc.tile([128, S], FP32, tag="sc", name="sc")
                                lo = sub * 64
                                nc.tensor.matmul(pss[:], qT[j][lo:lo + 64, ts(qt, 128)],
                                                 kT[j][lo:lo + 64, :], start=True, stop=True)
                                m = st_p.tile([128, 1], FP32, tag="m", name="m")
                                nc.vector.reduce_max(m[:], pss[:], axis=mybir.AxisListType.X)
                                nm = st_p.tile([128, 1], FP32, tag="nm", name="nm")
                                nc.vector.tensor_scalar_mul(nm[:], m[:], -0.125)
                                e = e_p.tile([128, S], BF16, tag="e", name="e")
                                zz = st_p.tile([128, 1], FP32, tag="zz", name="zz")
                                nc.scalar.activation(e[:], pss[:], AF.Exp,
                                                     bias=nm[:], scale=0.125,
                                                     accum_out=zz[:])
                                rz = st_p.tile([128, 1], FP32, tag="rz", name="rz")
                                nc.vector.reciprocal(rz[:], zz[:])
                                en = e_p.tile([128, S], BF16, tag="en", name="en")
                                nc.vector.tensor_scalar_mul(en[:], e[:], rz[:])
                                E[sub][qt] = en
                        AT = [[None] * NT for _ in range(2)]
                        for sub in range(2):
                            for kt2 in range(NT):
                                pst = ps_tr.tile([128, S], BF16, tag="tr", name="tr")
                                for qt in range(NT):
                                    nc.tensor.transpose(pst[:, ts(qt, 128)],
                                                        E[sub][qt][:, ts(kt2, 128)],
                                                        ident[:])
                                at = at_p.tile([128, S], BF16, tag="at", name="at")
                                if (sub + kt2) % 2 == 0:
                                    nc.scalar.copy(at[:], pst[:])
                                else:
                                    nc.vector.tensor_copy(at[:], pst[:])
                                AT[sub][kt2] = at
                        psoA = ps_sc.tile([128, S], FP32, tag="sc", name="sc_oA")
                        psoB = ps_sc.tile([128, S], FP32, tag="sc", name="sc_oB")
                        for kt2 in range(NT):
                            c0 = (2 * j) * DH
                            nc.tensor.matmul(psoA[0:64, :], vN[kt2][:, c0:c0 + 64],
                                             AT[0][kt2][:], start=(kt2 == 0), stop=(kt2 == NT - 1),
                                             tile_position=(0, 0))
                            c1 = (2 * j + 1) * DH
                            nc.tensor.matmul(psoB[64:128, :], vN[kt2][:, c1:c1 + 64],
                                             AT[1][kt2][:], start=(kt2 == 0), stop=(kt2 == NT - 1),
                                             tile_position=(0, 64))
                        ot = ot_p.tile([128, S], BF16, tag="ot", name="ot")
                        nc.vector.tensor_copy(ot[0:64, :], psoA[0:64, :])
                        nc.vector.tensor_copy(ot[64:128, :], psoB[64:128, :])
                        oT.append(ot)
                    else:
                        # post-LN scores are small: direct [k, q] layout, no
                        # max subtraction; Z via ones-matmul; 1/Z broadcast on
                        # gpsimd; normalization fused into the oT psum drain.
                        ET = [[None] * NT for _ in range(2)]
                        for kt2 in range(NT):
                            for sub in range(2):
                                psS = ps_sc.tile([128, S], FP32, tag="sc", name="sc")
                                lo = sub * 64
                                nc.tensor.matmul(psS[:], kT[j][lo:lo + 64, ts(kt2, 128)],
                                                 qT[j][lo:lo + 64, :], start=True, stop=True)
                                e = e_p.tile([128, S], BF16, tag="e", name="e")
                                nc.scalar.activation(e[:], psS[:], AF.Exp, scale=0.125)
                                ET[sub][kt2] = e
                        rzb = rzb_p.tile([128, S], FP32, tag="rzb", name="rzb")
                        for sub in range(2):
                            psZ = ps_tr.tile([128, S], FP32, tag="tr", name="tr_z")
                            for kt2 in range(NT):
                                nc.tensor.matmul(psZ[0:1, :], ones_b[:],
                                                 ET[sub][kt2][:], start=(kt2 == 0),
                                                 stop=(kt2 == NT - 1))
                            zrow = st_p.tile([1, S], FP32, tag="zrow", name="zrow", bufs=2)
                            nc.scalar.copy(zrow[:], psZ[0:1, :])
                            rzrow = st_p.tile([1, S], FP32, tag="rzrow", name="rzrow", bufs=2)
                            nc.vector.reciprocal(rzrow[:], zrow[:])
                            nc.gpsimd.partition_broadcast(
                                rzb[sub * 64:(sub + 1) * 64, :], rzrow[:], channels=64)
                        psoA = ps_sc.tile([128, S], FP32, tag="sc", name="sc_oA")
                        psoB = ps_sc.tile([128, S], FP32, tag="sc", name="sc_oB")
                        for kt2 in range(NT):
                            c0 = (2 * j) * DH
                            nc.tensor.matmul(psoA[0:64, :], vN[kt2][:, c0:c0 + 64],
                                             ET[0][kt2][:], start=(kt2 == 0), stop=(kt2 == NT - 1),
                                             tile_position=(0, 0))
                            c1 = (2 * j + 1) * DH
                            nc.tensor.matmul(psoB[64:128, :], vN[kt2][:, c1:c1 + 64],
                                             ET[1][kt2][:], start=(kt2 == 0), stop=(kt2 == NT - 1),
                                             tile_position=(0, 64))
                        ot = ot_p.tile([128, S], BF16, tag="ot", name="ot")
                        nc.vector.tensor_mul(ot[0:64, :], psoA[0:64, :], rzb[0:64, :])
                        nc.vector.tensor_mul(ot[64:128, :], psoB[64:128, :], rzb[64:128, :])
                        oT.append(ot)

                # ---- attn out proj + residual + LN ----
                def layer_norm_block(ps_in, t, g_sb, beta_sb):
                    z = z_p.tile([128, C], FP32, tag="z", name="z")
                    nc.vector.tensor_add(z[:], ps_in[:], hsc[s][t][:])
                    st6 = st_p.tile([128, 6], FP32, tag="st6", name="st6")
                    nc.vector.bn_stats(st6[:], z[:])
                    mv = st_p.tile([128, 2], FP32, tag="mv", name="mv")
                    nc.vector.bn_aggr(mv[:], st6[:])
                    rstd = st_p.tile([128, 1], FP32, tag="rstd", name="rstd")
                    nc.scalar.activation(rstd[:], mv[:, 1:2], AF.Sqrt, bias=eps_t[:])
                    nc.vector.reciprocal(rstd[:], rstd[:])
                    hn = hsc_p.tile([128, C], FP32, tag="hsc", name="hsc")
                    nc.vector.tensor_scalar(hn[:], z[:], scalar1=mv[:, 0:1],
                                            scalar2=rstd[:], op0=ALU.subtract,
                                            op1=ALU.mult)
                    if g_sb is not None:
                        nc.vector.tensor_mul(hn[:], hn[:], g_sb[:])
                    if beta_sb is not None:
                        nc.vector.tensor_add(hn[:], hn[:], beta_sb[:])
                    hb = hb_p.tile([128, C], BF16, tag="hb", name="hb")
                    if g_sb is None and beta_sb is None:
                        nc.gpsimd.tensor_scalar(hb[:], z[:], scalar1=mv[:, 0:1],
                                                scalar2=rstd[:], op0=ALU.subtract,
                                                op1=ALU.mult)
                    else:
                        nc.gpsimd.tensor_copy(hb[:], hn[:])
                    return hn, hb

                hb1 = []
                for t in range(NT):
                    psa = ps_mm.tile([128, C], FP32, tag="mm", name="mm")
                    for ci in range(NT):
                        nc.tensor.matmul(psa[:], oT[ci][:, ts(t, 128)],
                                         wo_sb[:, ci, :], start=(ci == 0), stop=(ci == NT - 1))
                    if use_bo:
                        nc.vector.tensor_add(psa[:], psa[:], vec_sb["bo"][:])
                    hn, hb = layer_norm_block(psa, t, vec_sb.get("g1"), vec_sb.get("beta1"))
                    hsc[s][t] = hn
                    hb1.append(hb)
                hcs2 = transpose_to_cs(hb1, "hcs2")

                # ---- FFN ----
                F1 = []
                for ft in range(NFT):
                    ps1 = ps_mm.tile([128, S], FP32, tag="mm", name="mm")
                    for ci in range(NT):
                        nc.tensor.matmul(ps1[:], w1_sb[:, ci, ts(ft, 128)],
                                         hcs2[ci][:], start=(ci == 0), stop=(ci == NT - 1))
                    f1 = f1_p.tile([128, S], BF16, tag="f1", name="f1")
                    bias = b1_sb[:, ft:ft + 1] if use_b1 else 0.0
                    nc.scalar.activation(f1[:], ps1[:], AF.Relu, bias=bias)
                    F1.append(f1)
                hb2 = []
                for t in range(NT):
                    psF = ps_sc.tile([128, C], FP32, tag="sc", name="sc_f2")
                    for ft in range(NFT):
                        nc.tensor.matmul(psF[:], F1[ft][:, ts(t, 128)],
                                         w2_sb[:, ft, :], start=(ft == 0),
                                         stop=(ft == NFT - 1))
                    if use_b2:
                        nc.vector.tensor_add(psF[:], psF[:], vec_sb["b2"][:])
                    hn, hb = layer_norm_block(psF, t, vec_sb.get("g2"), vec_sb.get("beta2"))
                    hsc[s][t] = hn
                    hb2.append(hb)
                if l < NL - 1:
                    hcs[s] = transpose_to_cs(hb2, "hcs")
                else:
                    for t in range(NT):
                        yr = out_p.tile([128, C], BF16, tag="yr", name="yr")
                        nc.scalar.activation(yr[:], hsc[s][t][:], AF.Relu)
                        nc.sync.dma_start(hout_d[s, ts(t, 128), :], yr[:])

    nc.compile()
    return nc


def build_endlayer(n_cores=N_CORES, kshard=KSH):
    KT = kshard // 128
    nc = bacc.Bacc("TRN2", target_bir_lowering=False, debug=False,
                   num_devices=n_cores)
    hT_d = nc.dram_tensor("hT", [kshard, B], BF16, kind="ExternalInput").ap()
    we_d = nc.dram_tensor("we", [kshard, O], BF16, kind="ExternalInput").ap()
    out_d = nc.dram_tensor("out", [B, O], FP32, kind="ExternalOutput").ap()

    with tile.TileContext(nc) as tc, ExitStack() as ctx:
        hp = ctx.enter_context(tc.tile_pool(name="h", bufs=1))
        wp = ctx.enter_context(tc.tile_pool(name="wep", bufs=8))
        op = ctx.enter_context(tc.tile_pool(name="op", bufs=1))
        psp = ctx.enter_context(tc.tile_pool(name="ps", bufs=2, space="PSUM"))

        hT = hp.tile([128, KT, B], BF16)
        nc.sync.dma_start(hT[:], hT_d.rearrange("(t p) b -> p t b", p=128))
        ps = psp.tile([B, O], FP32)
        for t in range(KT):
            we = wp.tile([128, O], BF16, tag="we", name="we")
            nc.sync.dma_start(we[:], we_d[ts(t, 128), :])
            nc.tensor.matmul(ps[:], hT[:, t, :], we[:],
                             start=(t == 0), stop=(t == KT - 1))
        ob = op.tile([B, O], FP32)
        nc.vector.tensor_copy(ob[:], ps[:])
        nc.sync.dma_start(out_d[:], ob[:])
    nc.compile()
    return nc


def pe_table():
    pos = np.arange(S, dtype=np.float32)[:, None]
    ie = np.arange(0, C, 2, dtype=np.float32)
    sin = np.sin(pos / 10000.0 ** (2.0 * ie / C))
    cos = np.cos(pos / 10000.0 ** (2.0 * (ie + 1.0) / C))
    pe = np.zeros((S, C), np.float32)
    pe[:, 0::2] = sin
    pe[:, 1::2] = cos
    return pe


_CACHE = {}


def _get_nc1(flags):
    key = ("t", flags)
    if key not in _CACHE:
        _CACHE[key] = build_transformer(
            use_g1=flags[0], use_beta1=flags[1], use_g2=flags[2],
            use_beta2=flags[3], use_bo=flags[4], use_b1=flags[5],
            use_b2=flags[6])
    return _CACHE[key]


def _get_nc2():
    if "e" not in _CACHE:
        _CACHE["e"] = build_endlayer()
    return _CACHE["e"]


def _bf(a):
    return np.asarray(a).astype(ml_dtypes.bfloat16)


def prep_inputs(x, Wq, Wk, Wv, Wo, bo, g1, beta1, W1, b1, W2, b2, g2, beta2,
                We, be):
    """Host-side prep shared by kernel() and the test harness."""
    x = np.asarray(x, dtype=np.float32)
    h0 = (np.swapaxes(x, 1, 2) * math.sqrt(C) + pe_table()[None]).astype(np.float32)

    bo, b1, b2 = (np.asarray(a, np.float32) for a in (bo, b1, b2))
    g1, beta1 = (np.asarray(a, np.float32) for a in (g1, beta1))
    g2, beta2 = (np.asarray(a, np.float32) for a in (g2, beta2))
    flags = (bool((g1 != 1).any()), bool(beta1.any()), bool((g2 != 1).any()),
             bool(beta2.any()), bool(bo.any()), bool(b1.any()), bool(b2.any()))

    base = {"wq": _bf(Wq), "wk": _bf(Wk), "wv": _bf(Wv), "wo": _bf(Wo),
            "w1": _bf(W1), "w2": _bf(W2)}
    names = ("g1", "beta1", "g2", "beta2", "bo", "b1", "b2")
    vals = (g1, beta1, g2, beta2, bo, b1, b2)
    for nm, used, val in zip(names, flags, vals):
        if used:
            base[nm] = val
    in_maps1 = []
    for c in range(N_CORES):
        m = dict(base)
        m["h0"] = h0[c * SPC:(c + 1) * SPC]
        in_maps1.append(m)
    return flags, in_maps1


def prep_end(h_relu_bf16, We):
    """h_relu_bf16: [B, S, C] bf16. Returns in_maps for the end-layer launch."""
    We_bf = _bf(We)
    in_maps2 = []
    for c in range(N_CORES):
        hsl = h_relu_bf16[:, c * (S // N_CORES):(c + 1) * (S // N_CORES), :]
        hT = np.ascontiguousarray(hsl.reshape(B, KSH).T)
        we_sh = np.ascontiguousarray(We_bf[c * KSH:(c + 1) * KSH])
        in_maps2.append({"hT": hT, "we": we_sh})
    return in_maps2


def kernel(x, Wq, Wk, Wv, Wo, bo, g1, beta1, W1, b1, W2, b2, g2, beta2, We,
           be, **_unused):
    flags, in_maps1 = prep_inputs(x, Wq, Wk, Wv, Wo, bo, g1, beta1, W1, b1,
                                  W2, b2, g2, beta2, We, be)
    nc1 = _get_nc1(flags)
    res1 = run_bass_kernel_spmd(nc1, in_maps1, list(range(N_CORES)))
    h_relu = np.concatenate([res1.results[c]["hout"] for c in range(N_CORES)],
                            axis=0)

    in_maps2 = prep_end(h_relu, We)
    nc2 = _get_nc2()
    res2 = run_bass_kernel_spmd(nc2, in_maps2, list(range(N_CORES)))
    out = np.zeros((B, O), np.float32)
    for c in range(N_CORES):
        out += res2.results[c]["out"]
    out += np.asarray(be, np.float32)[None, :]
    return out


# revision 12
# speedup vs baseline: 70.9954x; 70.9954x over previous
"""Trainium2 Bass kernel for nn_AttentionModel_23304492548756.

Single-launch SPMD design over 8 NeuronCores:
 - 6-layer transformer data-parallel over batch (2 samples/core), weights
   replicated, bf16 matmuls with fp32 PSUM accumulation.
 - The huge end-layer weight We [C*S, O] is sharded over its contraction dim
   (1/8 per core). The relu(h) activations are resharded on-device with an
   AllToAll (each core sends each other core its 64-seq-position slice), then
   each core computes a partial [B, O] which the host sums (cheaper than an
   on-device AllReduce of 32KB).
Layout notes: the residual stream lives as [S, C] fp32 tiles (LayerNorm
reduces over the free dim); matmul stationary operands come from an [C, S]
bf16 transposed copy produced on the PE via transpose-mode.  Attention for
layers >= 1 computes scores directly in [k, q] layout (post-LN scores are
tiny, so exp needs no max subtraction); the softmax denominator comes from a
ones-vector matmul and is folded into the attention-output PSUM drain.
Layer 0 (pre-LN scale ~sqrt(C)) uses a max-subtracted softmax in [q, k]
layout with PE transposes.
"""
import math
from contextlib import ExitStack

import numpy as np
import ml_dtypes

import concourse.bass as bass
import concourse.tile as tile
from concourse import bacc, mybir
from concourse.bass import ts
from concourse.masks import make_identity
from concourse.bass_utils import run_bass_kernel_spmd

FP32 = mybir.dt.float32
BF16 = mybir.dt.bfloat16
AF = mybir.ActivationFunctionType
ALU = mybir.AluOpType

B = 16
C = 512
S = 512
H = 8
L = 6
FF = 2048
O = 512
DH = C // H
EPS = 1e-5
NT = 4           # C/128 = S/128 tiles
NFT = FF // 128  # 16
N_CORES = 8
SPC = B // N_CORES          # samples per core
SSH = S // N_CORES          # seq positions per core in the end layer
KSH = C * S // N_CORES      # end-layer contraction shard
NKT = KSH // 128            # 256 k-tiles in the end layer


def build_full(n_layers=L, n_samples=SPC, n_cores=N_CORES,
               use_g1=False, use_beta1=False, use_g2=False, use_beta2=False,
               use_bo=False, use_b1=False, use_b2=False, emit_hout=False):
    """DRAM inputs (per core):
      h0   [NS, S, C] f32     x^T*sqrt(C)+pe slice (2 samples)
      wq, wk, wv, wo [L, 128, NT*C] bf16   (host-relayout, contiguous lines)
      w1   [L, 128, NT*FF] bf16
      w2   [L, 128, NFT*C] bf16
      we   [KSH, O] bf16      per-core shard of We
      b1   [L, FF] f32 and bo/b2/g1/beta1/g2/beta2 [L, C] f32 when used
    Output: out [B, O] f32 partial (host sums the 8 partials and adds be).
    """
    NL, NS = n_layers, n_samples
    nc = bacc.Bacc("TRN2", target_bir_lowering=False, debug=False,
                   num_devices=n_cores)

    h0_d = nc.dram_tensor("h0", [NS, S, C], FP32, kind="ExternalInput").ap()
    wq_d = nc.dram_tensor("wq", [NL, 128, NT * C], BF16, kind="ExternalInput").ap()
    wk_d = nc.dram_tensor("wk", [NL, 128, NT * C], BF16, kind="ExternalInput").ap()
    wv_d = nc.dram_tensor("wv", [NL, 128, NT * C], BF16, kind="ExternalInput").ap()
    wo_d = nc.dram_tensor("wo", [NL, 128, NT * C], BF16, kind="ExternalInput").ap()
    w1_d = nc.dram_tensor("w1", [NL, 128, NT * FF], BF16, kind="ExternalInput").ap()
    w2_d = nc.dram_tensor("w2", [NL, 128, NFT * C], BF16, kind="ExternalInput").ap()
    we_d = nc.dram_tensor("we", [NKT // 4, 128, 4 * O], BF16, kind="ExternalInput").ap()
    b1_d = nc.dram_tensor("b1", [NL, FF], FP32, kind="ExternalInput").ap() if use_b1 else None
    vec_d = {}
    for name, used in (("bo", use_bo), ("b2", use_b2), ("g1", use_g1),
                       ("beta1", use_beta1), ("g2", use_g2), ("beta2", use_beta2)):
        if used:
            vec_d[name] = nc.dram_tensor(name, [NL, C], FP32, kind="ExternalInput").ap()
    out_d = nc.dram_tensor("out", [B, O], FP32, kind="ExternalOutput").ap()
    hout_d = (nc.dram_tensor("hout", [NS, S, C], BF16, kind="ExternalOutput").ap()
              if emit_hout else None)

    with tile.TileContext(nc) as tc, ExitStack() as ctx:
        const_p = ctx.enter_context(tc.tile_pool(name="const", bufs=1))
        wpool = ctx.enter_context(tc.tile_pool(name="w", bufs=2))
        bias_p = ctx.enter_context(tc.tile_pool(name="biasv", bufs=2))
        hsc_p = ctx.enter_context(tc.tile_pool(name="hsc", bufs=8))
        hb_p = ctx.enter_context(tc.tile_pool(name="hb", bufs=4))
        hcs_p = ctx.enter_context(tc.tile_pool(name="hcs", bufs=5))
        qkv_p = ctx.enter_context(tc.tile_pool(name="qkv", bufs=4))
        e_p = ctx.enter_context(tc.tile_pool(name="e", bufs=8))
        at_p = ctx.enter_context(tc.tile_pool(name="at", bufs=8))
        ot_p = ctx.enter_context(tc.tile_pool(name="ot", bufs=4))
        f1_p = ctx.enter_context(tc.tile_pool(name="f1", bufs=17))
        z_p = ctx.enter_context(tc.tile_pool(name="z", bufs=5))
        rzb_p = ctx.enter_context(tc.tile_pool(name="rzb", bufs=2))
        st_p = ctx.enter_context(tc.tile_pool(name="st", bufs=8))
        out_p = ctx.enter_context(tc.tile_pool(name="out", bufs=2))
        dram_p = ctx.enter_context(tc.tile_pool(name="dram", bufs=1, space="DRAM"))
        ld_p = ctx.enter_context(tc.tile_pool(name="ld", bufs=1))
        hT_p = ctx.enter_context(tc.tile_pool(name="hT", bufs=8))
        we_p = ctx.enter_context(tc.tile_pool(name="wep", bufs=6))

        ps_big = ctx.enter_context(tc.tile_pool(name="ps_big", bufs=3, space="PSUM"))
        ps_tr = ctx.enter_context(tc.tile_pool(name="ps_tr", bufs=2, space="PSUM"))

        ident = const_p.tile([128, 128], BF16)
        make_identity(nc, ident[:])
        eps_t = const_p.tile([128, 1], FP32)
        nc.vector.memset(eps_t[:], EPS)
        ones_b = const_p.tile([128, 1], BF16)
        nc.vector.memset(ones_b[:], 1.0)
        ones_r = const_p.tile([1, 128], BF16)
        nc.vector.memset(ones_r[:], 1.0)

        a2a_in = dram_p.tile([n_cores, NS, SSH, C], BF16, name="a2a_in")
        a2a_out = dram_p.tile([n_cores, NS, SSH, C], BF16, name="a2a_out")

        hsc = [[None] * NT for _ in range(NS)]
        hcs = [[None] * NT for _ in range(NS)]

        def transpose_to_cs(hb_tiles, tag):
            res = []
            for t in range(NT):
                pst = ps_tr.tile([128, S], BF16, tag="tr", name="tr")
                for u in range(NT):
                    nc.tensor.transpose(pst[:, ts(u, 128)],
                                        hb_tiles[u][:, ts(t, 128)], ident[:])
                dst = hcs_p.tile([128, S], BF16, tag=tag, name=tag)
                nc.scalar.copy(dst[:], pst[:])
                res.append(dst)
            return res

        for s in range(NS):
            hbt = []
            for t in range(NT):
                hsc[s][t] = hsc_p.tile([128, C], FP32, tag="hsc", name="hsc")
                nc.sync.dma_start(hsc[s][t][:], h0_d[s, ts(t, 128), :])
                hb = hb_p.tile([128, C], BF16, tag="hb", name="hb")
                nc.gpsimd.tensor_copy(hb[:], hsc[s][t][:])
                hbt.append(hb)
            hcs[s] = transpose_to_cs(hbt, "hcs")

        for l in range(NL):
            wq_sb = wpool.tile([128, NT, C], BF16, tag="wq", name="wq")
            wk_sb = wpool.tile([128, NT, C], BF16, tag="wk", name="wk")
            wv_sb = wpool.tile([128, NT, C], BF16, tag="wv", name="wv")
            wo_sb = wpool.tile([128, NT, C], BF16, tag="wo", name="wo")
            w1_sb = wpool.tile([128, NT, FF], BF16, tag="w1", name="w1", bufs=1)
            w2_sb = wpool.tile([128, NFT, C], BF16, tag="w2", name="w2", bufs=1)
            nc.sync.dma_start(wq_sb[:], wq_d[l].rearrange("p (ci c) -> p ci c", ci=NT))
            nc.sync.dma_start(wk_sb[:], wk_d[l].rearrange("p (ci c) -> p ci c", ci=NT))
            nc.sync.dma_start(wv_sb[:], wv_d[l].rearrange("p (ci c) -> p ci c", ci=NT))
            nc.sync.dma_start(wo_sb[:], wo_d[l].rearrange("p (ci c) -> p ci c", ci=NT))
            nc.sync.dma_start(w1_sb[:], w1_d[l].rearrange("p (ci f) -> p ci f", ci=NT))
            nc.sync.dma_start(w2_sb[:], w2_d[l].rearrange("p (ft c) -> p ft c", ft=NFT))
            if use_b1:
                b1_sb = bias_p.tile([128, NFT], FP32, tag="b1", name="b1")
                nc.sync.dma_start(b1_sb[:], b1_d[l].rearrange("(ft p) -> p ft", p=128))
            vec_sb = {}
            for name in vec_d:
                vb = bias_p.tile([128, C], FP32, tag=name, name=name)
                src = bass.AP(tensor=vec_d[name].tensor, offset=l * C,
                              ap=[[0, 128], [1, C]])
                nc.gpsimd.dma_start(vb[:], src)
                vec_sb[name] = vb

            for s in range(NS):
                # ---- QKV ----
                qT, kT, vN = [], [], []
                for t in range(NT):
                    psqk = ps_big.tile([128, 2, C], FP32, tag="big", name="big_qk")
                    for ci in range(NT):
                        nc.tensor.matmul(psqk[:, 0, :], wq_sb[:, ci, ts(t, 128)],
                                         hcs[s][ci][:], start=(ci == 0), stop=(ci == NT - 1))
                    for ci in range(NT):
                        nc.tensor.matmul(psqk[:, 1, :], wk_sb[:, ci, ts(t, 128)],
                                         hcs[s][ci][:], start=(ci == 0), stop=(ci == NT - 1))
                    qk = qkv_p.tile([128, 2, S], BF16, tag="qk", name="qk")
                    nc.scalar.copy(qk[:], psqk[:])
                    qT.append(qk[:, 0, :])
                    kT.append(qk[:, 1, :])

                    psv = ps_big.tile([128, 2, C], FP32, tag="big", name="big_v")
                    for ci in range(NT):
                        nc.tensor.matmul(psv[:, 0, :], hcs[s][ci][:, ts(t, 128)],
                                         wv_sb[:, ci, :], start=(ci == 0), stop=(ci == NT - 1))
                    vt = qkv_p.tile([128, C], BF16, tag="v", name="v")
                    nc.vector.tensor_copy(vt[:], psv[:, 0, :])
                    vN.append(vt)

                # ---- attention (head pairs at rows 0-63 / 64-127) ----
                oT = []
                for j in range(NT):
                    if l == 0:
                        # max-subtracted softmax in [q, k] layout + PE transpose
                        E = [[None] * NT for _ in range(2)]
                        for qt in range(NT):
                            zz2 = st_p.tile([128, 2], FP32, tag="zz2", name="zz2")
                            rz2 = st_p.tile([128, 2], FP32, tag="rz2", name="rz2")
                            es = []
                            pssp = ps_big.tile([128, 2, S], FP32, tag="big", name="big_sc0")
                            for sub in range(2):
                                lo = sub * 64
                                nc.tensor.matmul(pssp[:, sub, :],
                                                 qT[j][lo:lo + 64, ts(qt, 128)],
                                                 kT[j][lo:lo + 64, :], start=True, stop=True)
                            for sub in range(2):
                                m = st_p.tile([128, 1], FP32, tag="m", name="m")
                                nc.vector.reduce_max(m[:], pssp[:, sub, :],
                                                     axis=mybir.AxisListType.X)
                                nm = st_p.tile([128, 1], FP32, tag="nm", name="nm")
                                nc.vector.tensor_scalar_mul(nm[:], m[:], -0.125)
                                e = e_p.tile([128, S], BF16, tag="e", name="e", bufs=4)
                                nc.scalar.activation(e[:], pssp[:, sub, :], AF.Exp,
                                                     bias=nm[:], scale=0.125,
                                                     accum_out=zz2[:, sub:sub + 1])
                                es.append(e)
                            nc.vector.reciprocal(rz2[:], zz2[:])
                            for sub in range(2):
                                en = e_p.tile([128, S], BF16, tag="en", name="en", bufs=8)
                                nc.vector.tensor_scalar_mul(en[:], es[sub][:],
                                                            rz2[:, sub:sub + 1])
                                E[sub][qt] = en
                        AT = [[None] * NT for _ in range(2)]
                        for sub in range(2):
                            for kt2 in range(NT):
                                pst = ps_tr.tile([128, S], BF16, tag="tr", name="tr")
                                for qt in range(NT):
                                    nc.tensor.transpose(pst[:, ts(qt, 128)],
                                                        E[sub][qt][:, ts(kt2, 128)],
                                                        ident[:])
                                at = at_p.tile([128, S], BF16, tag="at", name="at")
                                if (sub + kt2) % 2 == 0:
                                    nc.scalar.copy(at[:], pst[:])
                                else:
                                    nc.vector.tensor_copy(at[:], pst[:])
                                AT[sub][kt2] = at
                        psop = ps_big.tile([128, 2, S], FP32, tag="big", name="big_o0")
                        for kt2 in range(NT):
                            c0 = (2 * j) * DH
                            nc.tensor.matmul(psop[0:64, 0, :], vN[kt2][:, c0:c0 + 64],
                                             AT[0][kt2][:], start=(kt2 == 0),
                                             stop=(kt2 == NT - 1), tile_position=(0, 0))
                            c1 = (2 * j + 1) * DH
                            nc.tensor.matmul(psop[64:128, 1, :], vN[kt2][:, c1:c1 + 64],
                                             AT[1][kt2][:], start=(kt2 == 0),
                                             stop=(kt2 == NT - 1), tile_position=(0, 64))
                        ot = ot_p.tile([128, S], BF16, tag="ot", name="ot")
                        nc.vector.tensor_copy(ot[0:64, :], psop[0:64, 0, :])
                        nc.vector.tensor_copy(ot[64:128, :], psop[64:128, 1, :])
                        oT.append(ot)
                    else:
                        # direct [k, q] scores; Z via ones-matmul; 1/Z bcast on
                        # gpsimd; normalization fused into the oT PSUM drain
                        ET = [[None] * NT for _ in range(2)]
                        for kt2 in range(NT):
                            psSp = ps_big.tile([128, 2, S], FP32, tag="big", name="big_sc")
                            for sub in range(2):
                                lo = sub * 64
                                nc.tensor.matmul(psSp[:, sub, :],
                                                 kT[j][lo:lo + 64, ts(kt2, 128)],
                                                 qT[j][lo:lo + 64, :], start=True, stop=True)
                            ep = e_p.tile([128, 2, S], BF16, tag="ep", name="ep", bufs=5)
                            nc.scalar.activation(ep[:], psSp[:], AF.Exp, scale=0.125)
                            ET[0][kt2] = ep[:, 0, :]
                            ET[1][kt2] = ep[:, 1, :]
                        rzb = rzb_p.tile([128, S], FP32, tag="rzb", name="rzb")
                        for sub in range(2):
                            psZ = ps_tr.tile([128, S], FP32, tag="tr", name="tr_z")
                            for kt2 in range(NT):
                                nc.tensor.matmul(psZ[0:1, :], ones_b[:],
                                                 ET[sub][kt2][:], start=(kt2 == 0),
                                                 stop=(kt2 == NT - 1))
                            zrow = st_p.tile([1, S], FP32, tag="zrow", name="zrow", bufs=2)
                            nc.scalar.copy(zrow[:], psZ[0:1, :])
                            rz_bf = st_p.tile([1, S], BF16, tag="rzbf", name="rzbf", bufs=2)
                            with nc.allow_low_precision(reason="1/Z bcast via bf16 matmul"):
                                nc.vector.reciprocal(rz_bf[:], zrow[:])
                            psB = ps_tr.tile([128, S], FP32, tag="tr", name="tr_b")
                            nc.tensor.matmul(psB[:], ones_r[:], rz_bf[:],
                                             start=True, stop=True)
                            lo = sub * 64
                            nc.scalar.copy(rzb[lo:lo + 64, :], psB[lo:lo + 64, :])
                        psop = ps_big.tile([128, 2, S], FP32, tag="big", name="big_o")
                        for kt2 in range(NT):
                            c0 = (2 * j) * DH
                            nc.tensor.matmul(psop[0:64, 0, :], vN[kt2][:, c0:c0 + 64],
                                             ET[0][kt2][:], start=(kt2 == 0),
                                             stop=(kt2 == NT - 1), tile_position=(0, 0))
                            c1 = (2 * j + 1) * DH
                            nc.tensor.matmul(psop[64:128, 1, :], vN[kt2][:, c1:c1 + 64],
                                             ET[1][kt2][:], start=(kt2 == 0),
                                             stop=(kt2 == NT - 1), tile_position=(0, 64))
                        ot = ot_p.tile([128, S], BF16, tag="ot", name="ot")
                        nc.vector.tensor_mul(ot[0:64, :], psop[0:64, 0, :], rzb[0:64, :])
                        nc.vector.tensor_mul(ot[64:128, :], psop[64:128, 1, :],
                                             rzb[64:128, :])
                        oT.append(ot)

                # ---- LN over a 4-tile group with batched stats ----
                def ln_phase(ps_tiles, g_sb, beta_sb):
                    zs, hns, hbs = [], [], []
                    rs4 = st_p.tile([128, NT], FP32, tag="rs4", name="rs4")
                    for t in range(NT):
                        z = z_p.tile([128, C], FP32, tag="z", name="z")
                        nc.vector.scalar_tensor_tensor(
                            z[:], ps_tiles[t], 1.0, hsc[s][t][:],
                            op0=ALU.mult, op1=ALU.add,
                            accum_out=rs4[:, t:t + 1])
                        zs.append(z)
                    nm4 = st_p.tile([128, NT], FP32, tag="nm4", name="nm4")
                    nc.vector.tensor_scalar_mul(nm4[:], rs4[:], -1.0 / C)
                    ssq4 = st_p.tile([128, NT], FP32, tag="ssq4", name="ssq4")
                    sq_scr = z_p.tile([128, C], BF16, tag="sqs", name="sqs", bufs=1)
                    for t in range(NT):
                        nc.scalar.activation(sq_scr[:], zs[t][:], AF.Square,
                                             bias=nm4[:, t:t + 1],
                                             accum_out=ssq4[:, t:t + 1])
                    sd4 = st_p.tile([128, NT], FP32, tag="sd4", name="sd4")
                    nc.scalar.activation(sd4[:], ssq4[:], AF.Sqrt, bias=eps_t[:],
                                         scale=1.0 / C)
                    nc.vector.reciprocal(sd4[:], sd4[:])
                    mean4 = st_p.tile([128, NT], FP32, tag="mean4", name="mean4")
                    nc.vector.tensor_scalar_mul(mean4[:], rs4[:], 1.0 / C)
                    for t in range(NT):
                        hn = hsc_p.tile([128, C], FP32, tag="hsc", name="hsc")
                        nc.vector.tensor_scalar(hn[:], zs[t][:],
                                                scalar1=mean4[:, t:t + 1],
                                                scalar2=sd4[:, t:t + 1],
                                                op0=ALU.subtract, op1=ALU.mult)
                        if g_sb is not None:
                            nc.vector.tensor_mul(hn[:], hn[:], g_sb[:])
                        if beta_sb is not None:
                            nc.vector.tensor_add(hn[:], hn[:], beta_sb[:])
                        hb = hb_p.tile([128, C], BF16, tag="hb", name="hb")
                        if g_sb is None and beta_sb is None:
                            nc.gpsimd.tensor_scalar(hb[:], zs[t][:],
                                                    scalar1=mean4[:, t:t + 1],
                                                    scalar2=sd4[:, t:t + 1],
                                                    op0=ALU.subtract, op1=ALU.mult)
                        else:
                            nc.gpsimd.tensor_copy(hb[:], hn[:])
                        hns.append(hn)
                        hbs.append(hb)
                    return hns, hbs

                # ---- attn out proj + residual + LN1 ----
                psa_l = []
                for tp in range(NT // 2):
                    psap = ps_big.tile([128, 2, C], FP32, tag="big", name="big_pr")
                    for half in range(2):
                        t = 2 * tp + half
                        for ci in range(NT):
                            nc.tensor.matmul(psap[:, half, :], oT[ci][:, ts(t, 128)],
                                             wo_sb[:, ci, :], start=(ci == 0),
                                             stop=(ci == NT - 1))
                        if use_bo:
                            nc.vector.tensor_add(psap[:, half, :], psap[:, half, :],
                                                 vec_sb["bo"][:])
                        psa_l.append(psap[:, half, :])
                hns, hb1 = ln_phase(psa_l, vec_sb.get("g1"), vec_sb.get("beta1"))
                hsc[s] = hns
                hcs2 = transpose_to_cs(hb1, "hcs2")

                # ---- FFN ----
                F1 = []
                for fp in range(NFT // 2):
                    ps1p = ps_big.tile([128, 2, S], FP32, tag="big", name="big_f1")
                    for half in range(2):
                        ft = 2 * fp + half
                        for ci in range(NT):
                            nc.tensor.matmul(ps1p[:, half, :],
                                             w1_sb[:, ci, ts(ft, 128)],
                                             hcs2[ci][:], start=(ci == 0),
                                             stop=(ci == NT - 1))
                    f1p = f1_p.tile([128, 2, S], BF16, tag="f1p", name="f1p", bufs=8)
                    if use_b1:
                        for half in range(2):
                            ft = 2 * fp + half
                            nc.scalar.activation(f1p[:, half, :], ps1p[:, half, :],
                                                 AF.Relu, bias=b1_sb[:, ft:ft + 1])
                    else:
                        nc.scalar.activation(f1p[:], ps1p[:], AF.Relu)
                    F1.append(f1p)
                psf_l = []
                for tp in range(NT // 2):
                    psFp = ps_big.tile([128, 2, C], FP32, tag="big", name="big_f2")
                    for half in range(2):
                        t = 2 * tp + half
                        for ft in range(NFT):
                            nc.tensor.matmul(psFp[:, half, :],
                                             F1[ft // 2][:, ft % 2, ts(t, 128)],
                                             w2_sb[:, ft, :], start=(ft == 0),
                                             stop=(ft == NFT - 1))
                        if use_b2:
                            nc.vector.tensor_add(psFp[:, half, :], psFp[:, half, :],
                                                 vec_sb["b2"][:])
                        psf_l.append(psFp[:, half, :])
                hns, hb2 = ln_phase(psf_l, vec_sb.get("g2"), vec_sb.get("beta2"))
                hsc[s] = hns
                if l < NL - 1:
                    hcs[s] = transpose_to_cs(hb2, "hcs")
                else:
                    for t in range(NT):
                        yr = out_p.tile([128, C], BF16, tag="yr", name="yr")
                        nc.scalar.activation(yr[:], hsc[s][t][:], AF.Relu)
                        nc.sync.dma_start(a2a_in[2 * t, s, :, :], yr[0:64, :])
                        nc.sync.dma_start(a2a_in[2 * t + 1, s, :, :], yr[64:128, :])
                        if emit_hout:
                            nc.sync.dma_start(hout_d[s, ts(t, 128), :], yr[:])

        # ======== reshard + end layer ========
        nc.gpsimd.collective_compute(
            "AllToAll", ALU.bypass, replica_groups=[list(range(n_cores))],
            ins=[a2a_in[:]], outs=[a2a_out[:]])

        # hT tiles: [128(k), 16(b)] built by PE transpose of [16, 128] chunks
        NG = NKT // 16                      # 16 groups of 16 k-tiles
        hT = []
        for g in range(NG):
            ld = ld_p.tile([16, 4, C], BF16, tag="ld", name="ld")
            nc.sync.dma_start(ld[:], a2a_out[:, :, g * 4:(g + 1) * 4, :]
                              .rearrange("i b s c -> (i b) s c"))
            pst = ps_tr.tile([128, 16, 16], BF16, tag="tr", name="tr_h")
            for u in range(16):
                nc.tensor.transpose(pst[:, u, :],
                                    ld[:, u // 4, (u % 4) * 128:(u % 4 + 1) * 128],
                                    ident[0:16, 0:16])
            ht = hT_p.tile([128, 16, 16], BF16, tag="hT", name="hT", bufs=16)
            nc.scalar.copy(ht[:], pst[:])
            hT.append(ht)

        psOp = ps_big.tile([128, 2, O], FP32, tag="big", name="big_end")
        psO = psOp[0:B, 0, :]
        for kg in range(NKT // 4):
            we4 = we_p.tile([128, 4, O], BF16, tag="we", name="we", bufs=2)
            nc.sync.dma_start(we4[:], we_d[kg].rearrange("p (u o) -> p u o", u=4))
            for u in range(4):
                kt = kg * 4 + u
                nc.tensor.matmul(psO, hT[kt // 16][:, kt % 16, :], we4[:, u, :],
                                 start=(kt == 0), stop=(kt == NKT - 1))
        ob = out_p.tile([B, O], FP32, tag="ob", name="ob", bufs=1)
        nc.vector.tensor_copy(ob[:], psO)
        nc.sync.dma_start(out_d[:], ob[:])

    nc.compile()
    return nc


def pe_table():
    pos = np.arange(S, dtype=np.float32)[:, None]
    ie = np.arange(0, C, 2, dtype=np.float32)
    sin = np.sin(pos / 10000.0 ** (2.0 * ie / C))
    cos = np.cos(pos / 10000.0 ** (2.0 * (ie + 1.0) / C))
    pe = np.zeros((S, C), np.float32)
    pe[:, 0::2] = sin
    pe[:, 1::2] = cos
    return pe


_CACHE = {}


def _get_nc(flags):
    if flags not in _CACHE:
        _CACHE[flags] = build_full(
            use_g1=flags[0], use_beta1=flags[1], use_g2=flags[2],
            use_beta2=flags[3], use_bo=flags[4], use_b1=flags[5],
            use_b2=flags[6])
    return _CACHE[flags]


def _bf(a):
    return np.asarray(a).astype(ml_dtypes.bfloat16)


def _relayout(w, inner):
    """[L, n*128, inner] -> [L, 128, n*inner] contiguous per-partition lines."""
    Ln, K, _ = w.shape
    n = K // 128
    return np.ascontiguousarray(
        w.reshape(Ln, n, 128, inner).transpose(0, 2, 1, 3).reshape(Ln, 128, n * inner))


def prep_inputs(x, Wq, Wk, Wv, Wo, bo, g1, beta1, W1, b1, W2, b2, g2, beta2,
                We, be):
    x = np.asarray(x, dtype=np.float32)
    h0 = (np.swapaxes(x, 1, 2) * math.sqrt(C) + pe_table()[None]).astype(np.float32)

    bo, b1, b2 = (np.asarray(a, np.float32) for a in (bo, b1, b2))
    g1, beta1 = (np.asarray(a, np.float32) for a in (g1, beta1))
    g2, beta2 = (np.asarray(a, np.float32) for a in (g2, beta2))
    flags = (bool((g1 != 1).any()), bool(beta1.any()), bool((g2 != 1).any()),
             bool(beta2.any()), bool(bo.any()), bool(b1.any()), bool(b2.any()))

    We_bf = _bf(We)
    base = {"wq": _relayout(_bf(Wq), C), "wk": _relayout(_bf(Wk), C),
            "wv": _relayout(_bf(Wv), C), "wo": _relayout(_bf(Wo), C),
            "w1": _relayout(_bf(W1), FF), "w2": _relayout(_bf(W2), C)}
    names = ("g1", "beta1", "g2", "beta2", "bo", "b1", "b2")
    vals = (g1, beta1, g2, beta2, bo, b1, b2)
    for nm, used, val in zip(names, flags, vals):
        if used:
            base[nm] = val
    in_maps = []
    for c in range(N_CORES):
        m = dict(base)
        m["h0"] = h0[c * SPC:(c + 1) * SPC]
        wsh = We_bf[c * KSH:(c + 1) * KSH]
        m["we"] = np.ascontiguousarray(
            wsh.reshape(NKT // 4, 4, 128, O).transpose(0, 2, 1, 3)
               .reshape(NKT // 4, 128, 4 * O))
        in_maps.append(m)
    return flags, in_maps


def kernel(x, Wq, Wk, Wv, Wo, bo, g1, beta1, W1, b1, W2, b2, g2, beta2, We,
           be, **_unused):
    flags, in_maps = prep_inputs(x, Wq, Wk, Wv, Wo, bo, g1, beta1, W1, b1,
                                 W2, b2, g2, beta2, We, be)
    nc = _get_nc(flags)
    res = run_bass_kernel_spmd(nc, in_maps, list(range(N_CORES)))
    out = np.zeros((B, O), np.float32)
    for c in range(N_CORES):
        out += res.results[c]["out"]
    out += np.asarray(be, np.float32)[None, :]
    return out


# revision 20
# speedup vs baseline: 89.4315x; 1.2597x over previous
"""Trainium2 Bass kernel for nn_AttentionModel_23304492548756.

Single-launch SPMD design over 8 NeuronCores:
 - 6-layer transformer data-parallel over batch (2 samples/core), weights
   replicated, bf16 matmuls with fp32 PSUM accumulation.
 - The huge end-layer weight We [C*S, O] is sharded over its contraction dim
   (1/8 per core). The relu(h) activations are resharded on-device with an
   AllToAll (each core sends each other core its 64-seq-position slice), then
   each core computes a partial [B, O] which the host sums (cheaper than an
   on-device AllReduce of 32KB).
Layout notes: the residual stream lives as [S, C] fp32 tiles (LayerNorm
reduces over the free dim); matmul stationary operands come from an [C, S]
bf16 transposed copy produced on the PE via transpose-mode.  Attention for
layers >= 1 computes scores directly in [k, q] layout (post-LN scores are
tiny, so exp needs no max subtraction); the softmax denominator comes from a
ones-vector matmul and is folded into the attention-output PSUM drain.
Layer 0 (pre-LN scale ~sqrt(C)) uses a max-subtracted softmax in [q, k]
layout with PE transposes.
"""
import math
from contextlib import ExitStack

import numpy as np
import ml_dtypes

import concourse.bass as bass
import concourse.tile as tile
from concourse import bacc, mybir
from concourse.bass import ts
from concourse.masks import make_identity
from concourse.bass_utils import run_bass_kernel_spmd

FP32 = mybir.dt.float32
BF16 = mybir.dt.bfloat16
AF = mybir.ActivationFunctionType
ALU = mybir.AluOpType

B = 16
C = 512
S = 512
H = 8
L = 6
FF = 2048
O = 512
DH = C // H
EPS = 1e-5
NT = 4           # C/128 = S/128 tiles
NFT = FF // 128  # 16
N_CORES = 8
SPC = B // N_CORES          # samples per core
SSH = S // N_CORES          # seq positions per core in the end layer
KSH = C * S // N_CORES      # end-layer contraction shard
NKT = KSH // 128            # 256 k-tiles in the end layer


def build_full(n_layers=L, n_samples=SPC, n_cores=N_CORES,
               use_g1=False, use_beta1=False, use_g2=False, use_beta2=False,
               use_bo=False, use_b1=False, use_b2=False, emit_hout=False):
    """DRAM inputs (per core):
      h0   [NS, S, C] f32     x^T*sqrt(C)+pe slice (2 samples)
      wq, wk, wv, wo [L, 128, NT*C] bf16   (host-relayout, contiguous lines)
      w1   [L, 128, NT*FF] bf16
      w2   [L, 128, NFT*C] bf16
      we   [KSH, O] bf16      per-core shard of We
      b1   [L, FF] f32 and bo/b2/g1/beta1/g2/beta2 [L, C] f32 when used
    Output: out [B, O] f32 partial (host sums the 8 partials and adds be).
    """
    NL, NS = n_layers, n_samples
    any_vec = (use_g1 or use_beta1 or use_g2 or use_beta2 or use_bo or
               use_b1 or use_b2)
    we_bufs = 1 if any_vec else 3
    nc = bacc.Bacc("TRN2", target_bir_lowering=False, debug=False,
                   num_devices=n_cores)

    h0_d = nc.dram_tensor("h0", [NS, S, C], FP32, kind="ExternalInput").ap()
    wq_d = nc.dram_tensor("wq", [NL, 128, NT * C], BF16, kind="ExternalInput").ap()
    wk_d = nc.dram_tensor("wk", [NL, 128, NT * C], BF16, kind="ExternalInput").ap()
    wv_d = nc.dram_tensor("wv", [NL, 128, NT * C], BF16, kind="ExternalInput").ap()
    wo_d = nc.dram_tensor("wo", [NL, 128, NT * C], BF16, kind="ExternalInput").ap()
    w1_d = nc.dram_tensor("w1", [NL, 128, NT * FF], BF16, kind="ExternalInput").ap()
    w2_d = nc.dram_tensor("w2", [NL, 128, NFT * C], BF16, kind="ExternalInput").ap()
    we_d = nc.dram_tensor("we", [NKT // 4, 128, 4 * O], BF16, kind="ExternalInput").ap()
    b1_d = nc.dram_tensor("b1", [NL, FF], FP32, kind="ExternalInput").ap() if use_b1 else None
    vec_d = {}
    for name, used in (("bo", use_bo), ("b2", use_b2), ("g1", use_g1),
                       ("beta1", use_beta1), ("g2", use_g2), ("beta2", use_beta2)):
        if used:
            vec_d[name] = nc.dram_tensor(name, [NL, C], FP32, kind="ExternalInput").ap()
    out_d = nc.dram_tensor("out", [B, O], FP32, kind="ExternalOutput").ap()
    hout_d = (nc.dram_tensor("hout", [NS, S, C], BF16, kind="ExternalOutput").ap()
              if emit_hout else None)

    with tile.TileContext(nc) as tc, ExitStack() as ctx:
        const_p = ctx.enter_context(tc.tile_pool(name="const", bufs=1))
        wpool = ctx.enter_context(tc.tile_pool(name="w", bufs=2))
        bias_p = ctx.enter_context(tc.tile_pool(name="biasv", bufs=1))
        hsc_p = ctx.enter_context(tc.tile_pool(name="hsc", bufs=8))
        hb_p = ctx.enter_context(tc.tile_pool(name="hb", bufs=4))
        hcs_p = ctx.enter_context(tc.tile_pool(name="hcs", bufs=5))
        qkv_p = ctx.enter_context(tc.tile_pool(name="qkv", bufs=4))
        e_p = ctx.enter_context(tc.tile_pool(name="e", bufs=8))
        at_p = ctx.enter_context(tc.tile_pool(name="at", bufs=8 if not any_vec else 7))
        ot_p = ctx.enter_context(tc.tile_pool(name="ot", bufs=4))
        f1_p = ctx.enter_context(tc.tile_pool(name="f1", bufs=17))
        z_p = ctx.enter_context(tc.tile_pool(name="z", bufs=5))
        rzb_p = ctx.enter_context(tc.tile_pool(name="rzb", bufs=2))
        st_p = ctx.enter_context(tc.tile_pool(name="st", bufs=8))
        out_p = ctx.enter_context(tc.tile_pool(name="out", bufs=2 if not any_vec else 1))
        dram_p = ctx.enter_context(tc.tile_pool(name="dram", bufs=1, space="DRAM"))
        ld_p = ctx.enter_context(tc.tile_pool(name="ld", bufs=1))
        hT_p = ctx.enter_context(tc.tile_pool(name="hT", bufs=8))
        we_p = ctx.enter_context(tc.tile_pool(name="wep", bufs=6))

        ps_big = ctx.enter_context(tc.tile_pool(name="ps_big", bufs=3, space="PSUM"))
        ps_tr = ctx.enter_context(tc.tile_pool(name="ps_tr", bufs=2, space="PSUM"))

        ident = const_p.tile([128, 128], BF16)
        make_identity(nc, ident[:])
        eps_t = const_p.tile([128, 1], FP32)
        nc.vector.memset(eps_t[:], EPS)
        ones_b = const_p.tile([128, 1], BF16)
        nc.vector.memset(ones_b[:], 1.0)
        ones_r = const_p.tile([1, 128], BF16)
        nc.vector.memset(ones_r[:], 1.0)

        a2a_in = dram_p.tile([n_cores, NS, SSH, C], BF16, name="a2a_in")
        a2a_out = dram_p.tile([n_cores, NS, SSH, C], BF16, name="a2a_out")

        hsc = [[None] * NT for _ in range(NS)]
        hcs = [[None] * NT for _ in range(NS)]

        def transpose_to_cs(hb_tiles, tag):
            res = []
            for t in range(NT):
                pst = ps_tr.tile([128, S], BF16, tag="tr", name="tr")
                for u in range(NT):
                    nc.tensor.transpose(pst[:, ts(u, 128)],
                                        hb_tiles[u][:, ts(t, 128)], ident[:])
                dst = hcs_p.tile([128, S], BF16, tag=tag, name=tag)
                nc.vector.tensor_copy(dst[:], pst[:])
                res.append(dst)
            return res

        for s in range(NS):
            hbt = []
            for t in range(NT):
                hsc[s][t] = hsc_p.tile([128, C], FP32, tag="hsc", name="hsc")
                nc.sync.dma_start(hsc[s][t][:], h0_d[s, ts(t, 128), :])
                hb = hb_p.tile([128, C], BF16, tag="hb", name="hb")
                nc.gpsimd.tensor_copy(hb[:], hsc[s][t][:])
                hbt.append(hb)
            hcs[s] = transpose_to_cs(hbt, "hcs")

        for l in range(NL):
            wq_sb = wpool.tile([128, NT, C], BF16, tag="wq", name="wq")
            wk_sb = wpool.tile([128, NT, C], BF16, tag="wk", name="wk")
            wv_sb = wpool.tile([128, NT, C], BF16, tag="wv", name="wv")
            wo_sb = wpool.tile([128, NT, C], BF16, tag="wo", name="wo")
            w1_sb = wpool.tile([128, NT, FF], BF16, tag="w1", name="w1", bufs=1)
            w2_sb = wpool.tile([128, NFT, C], BF16, tag="w2", name="w2", bufs=1)
            nc.sync.dma_start(wq_sb[:], wq_d[l].rearrange("p (ci c) -> p ci c", ci=NT))
            nc.sync.dma_start(wk_sb[:], wk_d[l].rearrange("p (ci c) -> p ci c", ci=NT))
            nc.sync.dma_start(wv_sb[:], wv_d[l].rearrange("p (ci c) -> p ci c", ci=NT))
            nc.sync.dma_start(wo_sb[:], wo_d[l].rearrange("p (ci c) -> p ci c", ci=NT))
            nc.sync.dma_start(w1_sb[:], w1_d[l].rearrange("p (ci f) -> p ci f", ci=NT))
            nc.sync.dma_start(w2_sb[:], w2_d[l].rearrange("p (ft c) -> p ft c", ft=NFT))
            if use_b1:
                b1_sb = bias_p.tile([128, NFT], FP32, tag="b1", name="b1")
                nc.sync.dma_start(b1_sb[:], b1_d[l].rearrange("(ft p) -> p ft", p=128))
            vec_sb = {}
            for name in vec_d:
                vb = bias_p.tile([128, C], FP32, tag=name, name=name)
                src = bass.AP(tensor=vec_d[name].tensor, offset=l * C,
                              ap=[[0, 128], [1, C]])
                nc.gpsimd.dma_start(vb[:], src)
                vec_sb[name] = vb

            for s in range(NS):
                # ---- QKV ----
                qT, kT, vN = [], [], []
                for t in range(NT):
                    psqk = ps_big.tile([128, 2, C], FP32, tag="big", name="big_qk")
                    for ci in range(NT):
                        nc.tensor.matmul(psqk[:, 0, :], wq_sb[:, ci, ts(t, 128)],
                                         hcs[s][ci][:], start=(ci == 0), stop=(ci == NT - 1))
                    for ci in range(NT):
                        nc.tensor.matmul(psqk[:, 1, :], wk_sb[:, ci, ts(t, 128)],
                                         hcs[s][ci][:], start=(ci == 0), stop=(ci == NT - 1))
                    qk = qkv_p.tile([128, 2, S], BF16, tag="qk", name="qk")
                    nc.scalar.copy(qk[:], psqk[:])
                    qT.append(qk[:, 0, :])
                    kT.append(qk[:, 1, :])

                    psv = ps_big.tile([128, 2, C], FP32, tag="big", name="big_v")
                    for ci in range(NT):
                        nc.tensor.matmul(psv[:, 0, :], hcs[s][ci][:, ts(t, 128)],
                                         wv_sb[:, ci, :], start=(ci == 0), stop=(ci == NT - 1))
                    vt = qkv_p.tile([128, C], BF16, tag="v", name="v")
                    nc.vector.tensor_copy(vt[:], psv[:, 0, :])
                    vN.append(vt)

                # ---- attention (head pairs at rows 0-63 / 64-127) ----
                oT = []
                for j in range(NT):
                    if l == 0:
                        # max-subtracted softmax in [q, k] layout + PE transpose
                        E = [[None] * NT for _ in range(2)]
                        for qt in range(NT):
                            zz2 = st_p.tile([128, 2], FP32, tag="zz2", name="zz2")
                            rz2 = st_p.tile([128, 2], FP32, tag="rz2", name="rz2")
                            es = []
                            pssp = ps_big.tile([128, 2, S], FP32, tag="big", name="big_sc0")
                            for sub in range(2):
                                lo = sub * 64
                                nc.tensor.matmul(pssp[:, sub, :],
                                                 qT[j][lo:lo + 64, ts(qt, 128)],
                                                 kT[j][lo:lo + 64, :], start=True, stop=True)
                            for sub in range(2):
                                m = st_p.tile([128, 1], FP32, tag="m", name="m")
                                nc.vector.reduce_max(m[:], pssp[:, sub, :],
                                                     axis=mybir.AxisListType.X)
                                nm = st_p.tile([128, 1], FP32, tag="nm", name="nm")
                                nc.vector.tensor_scalar_mul(nm[:], m[:], -0.125)
                                e = e_p.tile([128, S], BF16, tag="e", name="e", bufs=4)
                                nc.scalar.activation(e[:], pssp[:, sub, :], AF.Exp,
                                                     bias=nm[:], scale=0.125,
                                                     accum_out=zz2[:, sub:sub + 1])
                                es.append(e)
                            nc.vector.reciprocal(rz2[:], zz2[:])
                            for sub in range(2):
                                en = e_p.tile([128, S], BF16, tag="en", name="en", bufs=8 if not any_vec else 7)
                                nc.vector.tensor_scalar_mul(en[:], es[sub][:],
                                                            rz2[:, sub:sub + 1])
                                E[sub][qt] = en
                        AT = [[None] * NT for _ in range(2)]
                        for sub in range(2):
                            for kt2 in range(NT):
                                pst = ps_tr.tile([128, S], BF16, tag="tr", name="tr")
                                for qt in range(NT):
                                    nc.tensor.transpose(pst[:, ts(qt, 128)],
                                                        E[sub][qt][:, ts(kt2, 128)],
                                                        ident[:])
                                at = at_p.tile([128, S], BF16, tag="at", name="at")
                                if (sub + kt2) % 2 == 0:
                                    nc.scalar.copy(at[:], pst[:])
                                else:
                                    nc.vector.tensor_copy(at[:], pst[:])
                                AT[sub][kt2] = at
                        psop = ps_big.tile([128, 2, S], FP32, tag="big", name="big_o0")
                        for kt2 in range(NT):
                            c0 = (2 * j) * DH
                            nc.tensor.matmul(psop[0:64, 0, :], vN[kt2][:, c0:c0 + 64],
                                             AT[0][kt2][:], start=(kt2 == 0),
                                             stop=(kt2 == NT - 1), tile_position=(0, 0))
                            c1 = (2 * j + 1) * DH
                            nc.tensor.matmul(psop[64:128, 1, :], vN[kt2][:, c1:c1 + 64],
                                             AT[1][kt2][:], start=(kt2 == 0),
                                             stop=(kt2 == NT - 1), tile_position=(0, 64))
                        ot = ot_p.tile([128, S], BF16, tag="ot", name="ot")
                        nc.vector.tensor_copy(ot[0:64, :], psop[0:64, 0, :])
                        nc.vector.tensor_copy(ot[64:128, :], psop[64:128, 1, :])
                        oT.append(ot)
                    else:
                        # direct [k, q] scores; Z via ones-matmul; 1/Z bcast on
                        # gpsimd; normalization fused into the oT PSUM drain
                        ET = [[None] * NT for _ in range(2)]
                        for kt2 in range(NT):
                            psSp = ps_big.tile([128, 2, S], FP32, tag="big", name="big_sc")
                            for sub in range(2):
                                lo = sub * 64
                                nc.tensor.matmul(psSp[:, sub, :],
                                                 kT[j][lo:lo + 64, ts(kt2, 128)],
                                                 qT[j][lo:lo + 64, :], start=True, stop=True)
                            ep = e_p.tile([128, 2, S], BF16, tag="ep", name="ep", bufs=5 if not any_vec else 4)
                            nc.scalar.activation(ep[:], psSp[:], AF.Exp, scale=0.125)
                            ET[0][kt2] = ep[:, 0, :]
                            ET[1][kt2] = ep[:, 1, :]
                        rzb = rzb_p.tile([128, S], FP32, tag="rzb", name="rzb")
                        for sub in range(2):
                            psZ = ps_tr.tile([128, S], FP32, tag="tr", name="tr_z")
                            for kt2 in range(NT):
                                nc.tensor.matmul(psZ[0:1, :], ones_b[:],
                                                 ET[sub][kt2][:], start=(kt2 == 0),
                                                 stop=(kt2 == NT - 1))
                            zrow = st_p.tile([1, S], FP32, tag="zrow", name="zrow", bufs=2)
                            nc.scalar.copy(zrow[:], psZ[0:1, :])
                            rz_bf = st_p.tile([1, S], BF16, tag="rzbf", name="rzbf", bufs=2)
                            with nc.allow_low_precision(reason="1/Z bcast via bf16 matmul"):
                                nc.vector.reciprocal(rz_bf[:], zrow[:])
                            psB = ps_tr.tile([128, S], FP32, tag="tr", name="tr_b")
                            nc.tensor.matmul(psB[:], ones_r[:], rz_bf[:],
                                             start=True, stop=True)
                            lo = sub * 64
                            nc.vector.tensor_copy(rzb[lo:lo + 64, :], psB[lo:lo + 64, :])
                        psop = ps_big.tile([128, 2, S], FP32, tag="big", name="big_o")
                        for kt2 in range(NT):
                            c0 = (2 * j) * DH
                            nc.tensor.matmul(psop[0:64, 0, :], vN[kt2][:, c0:c0 + 64],
                                             ET[0][kt2][:], start=(kt2 == 0),
                                             stop=(kt2 == NT - 1), tile_position=(0, 0))
                            c1 = (2 * j + 1) * DH
                            nc.tensor.matmul(psop[64:128, 1, :], vN[kt2][:, c1:c1 + 64],
                                             ET[1][kt2][:], start=(kt2 == 0),
                                             stop=(kt2 == NT - 1), tile_position=(0, 64))
                        ot = ot_p.tile([128, S], BF16, tag="ot", name="ot")
                        nc.vector.tensor_mul(ot[0:64, :], psop[0:64, 0, :], rzb[0:64, :])
                        nc.vector.tensor_mul(ot[64:128, :], psop[64:128, 1, :],
                                             rzb[64:128, :])
                        oT.append(ot)

                # ---- LN over a 4-tile group with batched stats ----
                def ln_phase(ps_tiles, g_sb, beta_sb):
                    zs, hns, hbs = [], [], []
                    rs4 = st_p.tile([128, NT], FP32, tag="rs4", name="rs4")
                    for t in range(NT):
                        z = z_p.tile([128, C], FP32, tag="z", name="z")
                        nc.vector.scalar_tensor_tensor(
                            z[:], ps_tiles[t], 1.0, hsc[s][t][:],
                            op0=ALU.mult, op1=ALU.add,
                            accum_out=rs4[:, t:t + 1])
                        zs.append(z)
                    nm4 = st_p.tile([128, NT], FP32, tag="nm4", name="nm4")
                    nc.vector.tensor_scalar_mul(nm4[:], rs4[:], -1.0 / C)
                    ssq4 = st_p.tile([128, NT], FP32, tag="ssq4", name="ssq4")
                    sq_scr = z_p.tile([128, C], BF16, tag="sqs", name="sqs", bufs=1)
                    for t in range(NT):
                        nc.scalar.activation(sq_scr[:], zs[t][:], AF.Square,
                                             bias=nm4[:, t:t + 1],
                                             accum_out=ssq4[:, t:t + 1])
                    sd4 = st_p.tile([128, NT], FP32, tag="sd4", name="sd4")
                    nc.scalar.activation(sd4[:], ssq4[:], AF.Sqrt, bias=eps_t[:],
                                         scale=1.0 / C)
                    nc.vector.reciprocal(sd4[:], sd4[:])
                    mean4 = st_p.tile([128, NT], FP32, tag="mean4", name="mean4")
                    nc.vector.tensor_scalar_mul(mean4[:], rs4[:], 1.0 / C)
                    for t in range(NT):
                        hn = hsc_p.tile([128, C], FP32, tag="hsc", name="hsc")
                        nc.vector.tensor_scalar(hn[:], zs[t][:],
                                                scalar1=mean4[:, t:t + 1],
                                                scalar2=sd4[:, t:t + 1],
                                                op0=ALU.subtract, op1=ALU.mult)
                        if g_sb is not None:
                            nc.vector.tensor_mul(hn[:], hn[:], g_sb[:])
                        if beta_sb is not None:
                            nc.vector.tensor_add(hn[:], hn[:], beta_sb[:])
                        hb = hb_p.tile([128, C], BF16, tag="hb", name="hb")
                        if g_sb is None and beta_sb is None:
                            nc.gpsimd.tensor_scalar(hb[:], zs[t][:],
                                                    scalar1=mean4[:, t:t + 1],
                                                    scalar2=sd4[:, t:t + 1],
                                                    op0=ALU.subtract, op1=ALU.mult)
                        else:
                            nc.gpsimd.tensor_copy(hb[:], hn[:])
                        hns.append(hn)
                        hbs.append(hb)
                    return hns, hbs

                # ---- attn out proj + residual + LN1 ----
                psa_l = []
                for tp in range(NT // 2):
                    psap = ps_big.tile([128, 2, C], FP32, tag="big", name="big_pr")
                    for half in range(2):
                        t = 2 * tp + half
                        for ci in range(NT):
                            nc.tensor.matmul(psap[:, half, :], oT[ci][:, ts(t, 128)],
                                             wo_sb[:, ci, :], start=(ci == 0),
                                             stop=(ci == NT - 1))
                        if use_bo:
                            nc.vector.tensor_add(psap[:, half, :], psap[:, half, :],
                                                 vec_sb["bo"][:])
                        psa_l.append(psap[:, half, :])
                hns, hb1 = ln_phase(psa_l, vec_sb.get("g1"), vec_sb.get("beta1"))
                hsc[s] = hns
                hcs2 = transpose_to_cs(hb1, "hcs2")

                # ---- FFN ----
                F1 = []
                for fp in range(NFT // 2):
                    ps1p = ps_big.tile([128, 2, S], FP32, tag="big", name="big_f1")
                    for half in range(2):
                        ft = 2 * fp + half
                        for ci in range(NT):
                            nc.tensor.matmul(ps1p[:, half, :],
                                             w1_sb[:, ci, ts(ft, 128)],
                                             hcs2[ci][:], start=(ci == 0),
                                             stop=(ci == NT - 1))
                    f1p = f1_p.tile([128, 2, S], BF16, tag="f1p", name="f1p", bufs=8)
                    if use_b1:
                        for half in range(2):
                            ft = 2 * fp + half
                            nc.scalar.activation(f1p[:, half, :], ps1p[:, half, :],
                                                 AF.Relu, bias=b1_sb[:, ft:ft + 1])
                    else:
                        nc.scalar.activation(f1p[:], ps1p[:], AF.Relu)
                    F1.append(f1p)
                psf_l = []
                for tp in range(NT // 2):
                    psFp = ps_big.tile([128, 2, C], FP32, tag="big", name="big_f2")
                    for half in range(2):
                        t = 2 * tp + half
                        for ft in range(NFT):
                            nc.tensor.matmul(psFp[:, half, :],
                                             F1[ft // 2][:, ft % 2, ts(t, 128)],
                                             w2_sb[:, ft, :], start=(ft == 0),
                                             stop=(ft == NFT - 1))
                        if use_b2:
                            nc.vector.tensor_add(psFp[:, half, :], psFp[:, half, :],
                                                 vec_sb["b2"][:])
                        psf_l.append(psFp[:, half, :])
                hns, hb2 = ln_phase(psf_l, vec_sb.get("g2"), vec_sb.get("beta2"))
                hsc[s] = hns
                if l < NL - 1:
                    hcs[s] = transpose_to_cs(hb2, "hcs")
                else:
                    for t in range(NT):
                        yr = out_p.tile([128, C], BF16, tag="yr", name="yr")
                        nc.scalar.activation(yr[:], hsc[s][t][:], AF.Relu)
                        nc.sync.dma_start(a2a_in[2 * t, s, :, :], yr[0:64, :])
                        nc.sync.dma_start(a2a_in[2 * t + 1, s, :, :], yr[64:128, :])
                        if emit_hout:
                            nc.sync.dma_start(hout_d[s, ts(t, 128), :], yr[:])

        # ======== reshard + end layer ========
        nc.gpsimd.collective_compute(
            "AllToAll", ALU.bypass, replica_groups=[list(range(n_cores))],
            ins=[a2a_in[:]], outs=[a2a_out[:]])

        # hT tiles: [128(k), 16(b)] built by PE transpose of [16, 128] chunks
        NG = NKT // 16                      # 16 groups of 16 k-tiles
        hT = []
        for g in range(NG):
            ld = ld_p.tile([16, 4, C], BF16, tag="ld", name="ld")
            nc.sync.dma_start(ld[:], a2a_out[:, :, g * 4:(g + 1) * 4, :]
                              .rearrange("i b s c -> (i b) s c"))
            pst = ps_tr.tile([128, 16, 16], BF16, tag="tr", name="tr_h")
            for u in range(16):
                nc.tensor.transpose(pst[:, u, :],
                                    ld[:, u // 4, (u % 4) * 128:(u % 4 + 1) * 128],
                                    ident[0:16, 0:16])
            ht = hT_p.tile([128, 16, 16], BF16, tag="hT", name="hT", bufs=16)
            nc.scalar.copy(ht[:], pst[:])
            hT.append(ht)

        psOp = ps_big.tile([128, 2, O], FP32, tag="big", name="big_end")
        psO = psOp[0:B, 0, :]
        for kg in range(NKT // 4):
            we4 = we_p.tile([128, 4, O], BF16, tag="we", name="we", bufs=we_bufs)
            nc.sync.dma_start(we4[:], we_d[kg].rearrange("p (u o) -> p u o", u=4))
            for u in range(4):
                kt = kg * 4 + u
                nc.tensor.matmul(psO, hT[kt // 16][:, kt % 16, :], we4[:, u, :],
                                 start=(kt == 0), stop=(kt == NKT - 1))
        ob = out_p.tile([B, O], FP32, tag="ob", name="ob", bufs=1)
        nc.vector.tensor_copy(ob[:], psO)
        nc.sync.dma_start(out_d[:], ob[:])

    nc.compile()
    return nc


def pe_table():
    pos = np.arange(S, dtype=np.float32)[:, None]
    ie = np.arange(0, C, 2, dtype=np.float32)
    sin = np.sin(pos / 10000.0 ** (2.0 * ie / C))
    cos = np.cos(pos / 10000.0 ** (2.0 * (ie + 1.0) / C))
    pe = np.zeros((S, C), np.float32)
    pe[:, 0::2] = sin
    pe[:, 1::2] = cos
    return pe


_CACHE = {}


def _get_nc(flags):
    if flags not in _CACHE:
        _CACHE[flags] = build_full(
            use_g1=flags[0], use_beta1=flags[1], use_g2=flags[2],
            use_beta2=flags[3], use_bo=flags[4], use_b1=flags[5],
            use_b2=flags[6])
    return _CACHE[flags]


def _bf(a):
    return np.asarray(a).astype(ml_dtypes.bfloat16)


def _relayout(w, inner):
    """[L, n*128, inner] -> [L, 128, n*inner] contiguous per-partition lines."""
    Ln, K, _ = w.shape
    n = K // 128
    return np.ascontiguousarray(
        w.reshape(Ln, n, 128, inner).transpose(0, 2, 1, 3).reshape(Ln, 128, n * inner))


def prep_inputs(x, Wq, Wk, Wv, Wo, bo, g1, beta1, W1, b1, W2, b2, g2, beta2,
                We, be):
    x = np.asarray(x, dtype=np.float32)
    h0 = (np.swapaxes(x, 1, 2) * math.sqrt(C) + pe_table()[None]).astype(np.float32)

    bo, b1, b2 = (np.asarray(a, np.float32) for a in (bo, b1, b2))
    g1, beta1 = (np.asarray(a, np.float32) for a in (g1, beta1))
    g2, beta2 = (np.asarray(a, np.float32) for a in (g2, beta2))
    flags = (bool((g1 != 1).any()), bool(beta1.any()), bool((g2 != 1).any()),
             bool(beta2.any()), bool(bo.any()), bool(b1.any()), bool(b2.any()))

    We_bf = _bf(We)
    base = {"wq": _relayout(_bf(Wq), C), "wk": _relayout(_bf(Wk), C),
            "wv": _relayout(_bf(Wv), C), "wo": _relayout(_bf(Wo), C),
            "w1": _relayout(_bf(W1), FF), "w2": _relayout(_bf(W2), C)}
    names = ("g1", "beta1", "g2", "beta2", "bo", "b1", "b2")
    vals = (g1, beta1, g2, beta2, bo, b1, b2)
    for nm, used, val in zip(names, flags, vals):
        if used:
            base[nm] = val
    in_maps = []
    for c in range(N_CORES):
        m = dict(base)
        m["h0"] = h0[c * SPC:(c + 1) * SPC]
        wsh = We_bf[c * KSH:(c + 1) * KSH]
        m["we"] = np.ascontiguousarray(
            wsh.reshape(NKT // 4, 4, 128, O).transpose(0, 2, 1, 3)
               .reshape(NKT // 4, 128, 4 * O))
        in_maps.append(m)
    return flags, in_maps


def kernel(x, Wq, Wk, Wv, Wo, bo, g1, beta1, W1, b1, W2, b2, g2, beta2, We,
           be, **_unused):
    flags, in_maps = prep_inputs(x, Wq, Wk, Wv, Wo, bo, g1, beta1, W1, b1,
                                 W2, b2, g2, beta2, We, be)
    nc = _get_nc(flags)
    res = run_bass_kernel_spmd(nc, in_maps, list(range(N_CORES)))
    out = np.zeros((B, O), np.float32)
    for c in range(N_CORES):
        out += res.results[c]["out"]
    out += np.asarray(be, np.float32)[None, :]
    return out
